# revision 8
# baseline (speedup 1.0000x reference)
"""LoRA attention kernel for 8 trn2 NeuronCores, tensor-parallel over heads.

Sharding: core s owns heads 2s, 2s+1 (a 128-row slice of the HD=1024 dim).
Each core computes q/k/v projections (base + LoRA fused), attention for its
4 (batch, head) pairs, and a partial output projection; the host sums the 8
partials and adds b_out.

Layouts (per core, on-chip):
  xT   [C=1024, B*N=4096]   activations transposed (contraction dim C on
                            partitions, 8 chunks of 128)
  qT/kT/vT [128, 4096]      2 heads x 64 dims on partitions
  attention runs in S^T layout: S^T[k, q] = K^T.T @ Q^T per 128-key chunk,
  exp via ScalarE (mask folded in as a per-partition additive bias), then
  O^T accumulated with lhsT = [V | ones] so the softmax denominator falls
  out of the same matmuls as PSUM row 64.
"""

import numpy as np

import concourse.bass as bass
import concourse.tile as tile
from concourse import bacc, mybir
from concourse.bass_utils import run_bass_kernel_spmd

H, D, R, C, B, N = 16, 64, 10, 1024, 2, 2048
BN = B * N
SCALING = 1.0 / R
ATT_SCALE = float(D) ** -0.5
NCORES = 8
F32 = mybir.dt.float32
F32R = mybir.dt.float32r
NCH = BN // 512  # 8 n-chunks of 512
CCH = C // 128  # 8 contraction chunks
KCH = N // 128  # 16 key chunks per (b,h)
QCH = N // 512  # 4 query chunks per (b,h)


def build_nc(dbg=False):
    nc = bacc.Bacc("TRN2", target_bir_lowering=False, debug=False,
                   num_devices=NCORES)
    if dbg:
        dbg_q = nc.dram_tensor("dbg_q", [128, BN], F32, kind="ExternalOutput")
        dbg_k = nc.dram_tensor("dbg_k", [128, BN], F32, kind="ExternalOutput")
        dbg_v = nc.dram_tensor("dbg_v", [128, BN], F32, kind="ExternalOutput")
        dbg_ao = nc.dram_tensor("dbg_ao", [128, BN], F32, kind="ExternalOutput")
    xT = nc.dram_tensor("xT", [C, BN], F32R, kind="ExternalInput")
    wqT = nc.dram_tensor("wqT", [C, 128], F32R, kind="ExternalInput")
    wkT = nc.dram_tensor("wkT", [C, 128], F32R, kind="ExternalInput")
    wvT = nc.dram_tensor("wvT", [C, 128], F32R, kind="ExternalInput")
    aT = nc.dram_tensor("aT", [C, 64], F32R, kind="ExternalInput")
    bB = nc.dram_tensor("bB", [42, 256], F32R, kind="ExternalInput")
    bq = nc.dram_tensor("bq", [128, 1], F32, kind="ExternalInput")
    bv = nc.dram_tensor("bv", [128, 1], F32, kind="ExternalInput")
    woT = nc.dram_tensor("woT", [CCH, 128, 128], F32R, kind="ExternalInput")
    idn = nc.dram_tensor("idn", [128, 128], F32R, kind="ExternalInput")
    ones = nc.dram_tensor("ones", [128, KCH], F32R, kind="ExternalInput")
    mb = nc.dram_tensor("mb", [128, B * KCH], F32, kind="ExternalInput")
    yT = nc.dram_tensor("yT", [CCH, 128, BN], F32, kind="ExternalOutput")

    with tile.TileContext(nc) as tc:
        with (
            tc.tile_pool(name="wts", bufs=1) as wts,
            tc.tile_pool(name="acts", bufs=1) as acts,
            tc.tile_pool(name="xin", bufs=3) as xin,
            tc.tile_pool(name="zt", bufs=2) as ztp,
            tc.tile_pool(name="pt", bufs=6) as ptp,
            tc.tile_pool(name="vsb", bufs=2) as vsbp,
            tc.tile_pool(name="rec", bufs=2) as recp,
            tc.tile_pool(name="rbc", bufs=2) as rbcp,
            tc.tile_pool(name="yout", bufs=4) as youtp,
            tc.tile_pool(name="ps_s", bufs=4, space="PSUM") as ps_s,
            tc.tile_pool(name="ps_o", bufs=2, space="PSUM") as ps_o,
            tc.tile_pool(name="ps_vt", bufs=1, space="PSUM") as ps_vt,
        ):
            # --- resident weights ---
            wq_s = wts.tile([128, CCH, 128], F32R)
            nc.sync.dma_start(wq_s[:], wqT.ap().rearrange("(i p) m -> p i m", p=128))
            wk_s = wts.tile([128, CCH, 128], F32R)
            nc.sync.dma_start(wk_s[:], wkT.ap().rearrange("(i p) m -> p i m", p=128))
            wv_s = wts.tile([128, CCH, 128], F32R)
            nc.sync.dma_start(wv_s[:], wvT.ap().rearrange("(i p) m -> p i m", p=128))
            a_s = wts.tile([128, CCH, 64], F32R)
            nc.sync.dma_start(a_s[:], aT.ap().rearrange("(i p) m -> p i m", p=128))
            bB_s = wts.tile([42, 256], F32R)
            nc.sync.dma_start(bB_s[:], bB.ap())
            bq_s = wts.tile([128, 1], F32)
            nc.sync.dma_start(bq_s[:], bq.ap())
            bv_s = wts.tile([128, 1], F32)
            nc.sync.dma_start(bv_s[:], bv.ap())
            wo_s = wts.tile([128, CCH, 128], F32R)
            nc.sync.dma_start(wo_s[:], woT.ap().rearrange("i p m -> p i m"))
            mb_s = wts.tile([128, B * KCH], F32)
            nc.sync.dma_start(mb_s[:], mb.ap())
            ident = wts.tile([128, 128], F32R)
            nc.sync.dma_start(ident[:], idn.ap())
            ones_s = wts.tile([128, KCH], F32R)
            nc.sync.dma_start(ones_s[:], ones.ap())

            # --- persistent activations ---
            qT = acts.tile([128, BN], F32R)
            kT = acts.tile([128, BN], F32R)
            vT = acts.tile([128, BN], F32R)
            aoT = acts.tile([128, BN], F32R)

            xT_r = xT.ap().rearrange("(i p) n -> p i n", p=128)

            # ---------- phase 1: projections ----------
            for nch in range(NCH):
                nsl = bass.ts(nch, 512)
                x_t = xin.tile([128, CCH, 512], F32R)
                nc.sync.dma_start(x_t[:], xT_r[:, :, nsl])

                z_ps = ps_o.tile([64, 512], F32, tag="o")
                for i in range(CCH):
                    nc.tensor.matmul(z_ps[:], (a_s[:, i, :]), (x_t[:, i, :]),
                                     start=(i == 0), stop=(i == CCH - 1))
                z_t = ztp.tile([64, 512], F32R)
                nc.vector.tensor_copy(z_t[:], z_ps[:])

                q_ps = ps_s.tile([128, 512], F32, tag="s")
                for i in range(CCH):
                    nc.tensor.matmul(q_ps[:], (wq_s[:, i, :]), (x_t[:, i, :]),
                                     start=(i == 0), stop=False)
                nc.tensor.matmul(q_ps[:], (bB_s[0:R, 0:128]), (z_t[0:R, :]),
                                 start=False, stop=True)
                nc.scalar.activation(qT[:, nsl], q_ps[:],
                                     mybir.ActivationFunctionType.Identity,
                                     bias=bq_s[:])

                k_ps = ps_s.tile([128, 512], F32, tag="s")
                for i in range(CCH):
                    nc.tensor.matmul(k_ps[:], (wk_s[:, i, :]), (x_t[:, i, :]),
                                     start=(i == 0), stop=(i == CCH - 1))
                nc.vector.tensor_copy(kT[:, nsl], k_ps[:])

                v_ps = ps_s.tile([128, 512], F32, tag="s")
                for i in range(CCH):
                    nc.tensor.matmul(v_ps[:], (wv_s[:, i, :]), (x_t[:, i, :]),
                                     start=(i == 0), stop=False)
                nc.tensor.matmul(v_ps[:], (bB_s[32:32 + R, 128:256]),
                                 (z_t[32:32 + R, :]), start=False, stop=True)
                nc.scalar.activation(vT[:, nsl], v_ps[:],
                                     mybir.ActivationFunctionType.Identity,
                                     bias=bv_s[:])

            # ---------- phase 2: attention ----------
            for b in range(B):
                for hh in range(2):
                    hsl = bass.ds(hh * 64, 64)
                    kb = b * N
                    vt_ps = ps_vt.tile([128, KCH, 64], F32R)
                    for kc in range(KCH):
                        nc.tensor.transpose(
                            vt_ps[:, kc, :],
                            vT[hsl, bass.ds(kb + kc * 128, 128)],
                            ident[hsl, hsl])
                    v_sb = vsbp.tile([128, KCH, 65], F32R)
                    nc.vector.tensor_copy(v_sb[:, :, 64:65], ones_s[:])
                    nc.vector.tensor_copy(v_sb[:, :, 0:64], vt_ps[:])

                    for qc in range(QCH):
                        qsl = bass.ds(kb + qc * 512, 512)
                        q_ap = qT[hsl, qsl]
                        o_ps = ps_o.tile([65, 512], F32, tag="o")
                        for kc in range(KCH):
                            s_ps = ps_s.tile([128, 512], F32, tag="s")
                            nc.tensor.matmul(
                                s_ps[:], (kT[hsl, bass.ds(kb + kc * 128, 128)]),
                                (q_ap), start=True, stop=True)
                            p_sb = ptp.tile([128, 512], F32R)
                            nc.scalar.activation(
                                p_sb[:], s_ps[:],
                                mybir.ActivationFunctionType.Exp,
                                bias=mb_s[:, bass.ds(b * KCH + kc, 1)],
                                scale=ATT_SCALE)
                            nc.tensor.matmul(o_ps[:], (v_sb[:, kc, :]),
                                             (p_sb[:]),
                                             start=(kc == 0), stop=(kc == KCH - 1))
                        rec = recp.tile([1, 512], F32)
                        nc.vector.reciprocal(rec[:], o_ps[64:65, :])
                        rbc = rbcp.tile([64, 512], F32)
                        nc.gpsimd.partition_broadcast(rbc[:], rec[:])
                        nc.vector.tensor_mul(aoT[hsl, qsl], o_ps[0:64, :], rbc[:])

            if dbg:
                nc.sync.dma_start(dbg_q.ap(), qT[:].bitcast(F32))
                nc.sync.dma_start(dbg_k.ap(), kT[:].bitcast(F32))
                nc.sync.dma_start(dbg_v.ap(), vT[:].bitcast(F32))
                nc.sync.dma_start(dbg_ao.ap(), aoT[:].bitcast(F32))

            # ---------- phase 3: output projection ----------
            for nch in range(NCH):
                nsl = bass.ts(nch, 512)
                for ci in range(CCH):
                    y_ps = ps_s.tile([128, 512], F32, tag="s")
                    nc.tensor.matmul(y_ps[:], (wo_s[:, ci, :]), (aoT[:, nsl]),
                                     start=True, stop=True)
                    y_sb = youtp.tile([128, 512], F32)
                    if ci % 2 == 0:
                        nc.scalar.copy(y_sb[:], y_ps[:])
                    else:
                        nc.vector.tensor_copy(y_sb[:], y_ps[:])
                    nc.sync.dma_start(yT.ap()[ci, :, nsl], y_sb[:])
    nc.compile()
    return nc


_NC = None


def _get_nc():
    global _NC
    if _NC is None:
        _NC = build_nc()
    return _NC


def _bB(Bq_sl, Bv_sl):
    out = np.zeros((42, 256), np.float32)
    out[0:R, 0:128] = (Bq_sl * SCALING).T
    out[32:32 + R, 128:256] = (Bv_sl * SCALING).T
    return out


def _prep_in_maps(inputs):
    x = np.asarray(inputs["x"], np.float32)
    mask = np.asarray(inputs["mask"])
    W_qkv = np.asarray(inputs["W_qkv"], np.float32)
    Wq_base = np.asarray(inputs["Wq_base"], np.float32)
    bq = np.asarray(inputs["bq"], np.float32)
    Aq = np.asarray(inputs["Aq"], np.float32)
    Bq = np.asarray(inputs["Bq"], np.float32)
    Wv_base = np.asarray(inputs["Wv_base"], np.float32)
    bv = np.asarray(inputs["bv"], np.float32)
    Av = np.asarray(inputs["Av"], np.float32)
    Bv = np.asarray(inputs["Bv"], np.float32)
    W_out = np.asarray(inputs["W_out"], np.float32)

    xT = np.ascontiguousarray(x.reshape(BN, C).T)
    Wq_eff = W_qkv[0:H * D] + Wq_base
    Wk = W_qkv[H * D:2 * H * D]
    Wv_eff = W_qkv[2 * H * D:3 * H * D] + Wv_base
    aT = np.zeros((C, 64), np.float32)
    aT[:, 0:R] = Aq.T
    aT[:, 32:32 + R] = Av.T
    mbias = np.where(mask.reshape(BN), 0.0, -1e5).astype(np.float32)
    mb = np.ascontiguousarray(mbias.reshape(B * KCH, 128).T)

    in_maps = []
    for s in range(NCORES):
        sl = slice(s * 128, (s + 1) * 128)
        in_maps.append({
            "xT": xT,
            "wqT": np.ascontiguousarray(Wq_eff[sl].T),
            "wkT": np.ascontiguousarray(Wk[sl].T),
            "wvT": np.ascontiguousarray(Wv_eff[sl].T),
            "aT": aT,
            "bB": _bB(Bq[sl], Bv[sl]),
            "bq": np.ascontiguousarray(bq[sl, None]),
            "bv": np.ascontiguousarray(bv[sl, None]),
            "woT": np.ascontiguousarray(
                W_out[:, sl].reshape(CCH, 128, 128).transpose(0, 2, 1)),
            "mb": mb,
            "idn": np.eye(128, dtype=np.float32),
            "ones": np.ones((128, KCH), np.float32),
        })
    return in_maps


def _assemble(results, b_out):
    acc = np.zeros((C, BN), np.float64)
    for r in results:
        acc += r["yT"].reshape(C, BN)
    out = acc.T.astype(np.float32) + np.asarray(b_out, np.float32)[None, :]
    return np.ascontiguousarray(out.reshape(B, N, C))


def kernel(**inputs):
    nc = _get_nc()
    in_maps = _prep_in_maps(inputs)
    res = run_bass_kernel_spmd(nc, in_maps, core_ids=list(range(NCORES)))
    return _assemble(res.results, inputs["b_out"])


def run_traced(inputs):
    """test harness hook: returns (output, exec_time_ns)."""
    nc = _get_nc()
    in_maps = _prep_in_maps(inputs)
    res = run_bass_kernel_spmd(nc, in_maps, core_ids=list(range(NCORES)),
                               trace=True)
    return _assemble(res.results, inputs["b_out"]), res.exec_time_ns
